# revision 1
# baseline (speedup 1.0000x reference)
"""Bezier-stroke rasterizer (AIR/Guide-style) as a Trainium2 Bass/Tile kernel.

Math: for each (batch, stroke k): control points -> Bezier curve at 500 ts ->
gaussian blob rasterization summed over t -> presence gating -> max-norm ->
tanh-norm -> sum over k -> tanh-norm.

Key factorization: exp(-inv*(dy^2+dx^2)) = exp(-inv*dy^2) * exp(-inv*dx^2), so
strokes[y,x] = sum_t ey[t,y] * ex[t,x]  ==  ey^T @ ex  (contraction over t on
the PE partition dim).  dx[t, (pair,g)] = curve_x[t,pair] - GRID[g] is itself
produced by a single PE matmul by augmenting the Bernstein basis with a ones
row and the rhs with a -GRID row.

Sharding: pure data parallel, 8 batches per core across 8 NeuronCores.
"""

import sys
import numpy as np
from math import comb, tanh

sys.path.insert(0, "/opt/trn_rl_repo")

from concourse import bass, bacc, tile, mybir  # noqa: E402
from concourse.bass_utils import run_bass_kernel_spmd  # noqa: E402

BS, K, PTS, RES, STEPS = 64, 4, 5, 28, 500
NCORES = 8
BL = BS // NCORES          # local batches per core = 8
NPAIR = BL * K             # (batch, stroke) pairs per core = 32
NCH = 4                    # t chunks
CL = STEPS // NCH          # 125 t per chunk
G = 112                    # 4 pairs * 28 rows per stroke-group
EPS = 1e-6
F32 = mybir.dt.float32
AF = mybir.ActivationFunctionType
ALU = mybir.AluOpType


def _host_consts():
    t = np.linspace(0.0, 1.0, STEPS, dtype=np.float32)[:, None]
    i = np.arange(PTS, dtype=np.float32)[None, :]
    binom = np.array([comb(PTS - 1, j) for j in range(PTS)], dtype=np.float32)[None, :]
    basis = binom * (t ** i) * ((1.0 - t) ** (PTS - 1 - i))       # [500, 5]
    grid = np.linspace(0.0, 1.0, RES, dtype=np.float32)           # [28]

    basT6 = np.ones((PTS + 1, STEPS), dtype=np.float32)
    basT6[:PTS] = basis.T                                          # row 5 = ones
    neggrid = -np.tile(grid, NPAIR)[None, :]                       # [1, 896]
    mask112 = np.kron(np.eye(4, dtype=np.float32), np.ones((RES, RES), np.float32))
    ident112 = np.eye(G, dtype=np.float32)
    sel4 = np.kron(np.eye(4, dtype=np.float32), np.ones((1, RES), np.float32))  # [4,112]
    ksum = np.kron(np.ones((4, 1), np.float32), np.eye(RES, dtype=np.float32))  # [112,28]
    return basT6, neggrid, mask112, ident112, sel4, ksum


def _build_program(sigma, slope_strk, slope, stage=99):
    inv = 1.0 / (2.0 * sigma * sigma)
    post1 = 1.0 / tanh(slope_strk)       # divide after stroke tanh (folded)
    post2 = 1.0 / tanh(slope)            # divide after final tanh

    nc = bacc.Bacc(None, target_bir_lowering=False)

    zw10_d = nc.dram_tensor("zw10", [NPAIR, 10], F32, kind="ExternalInput")
    zwh_d = nc.dram_tensor("zwh", [NPAIR, 3], F32, kind="ExternalInput")
    zpt_d = nc.dram_tensor("zpt", [K, BL], F32, kind="ExternalInput")
    bas_d = nc.dram_tensor("bast6", [PTS + 1, STEPS], F32, kind="ExternalInput")
    ngr_d = nc.dram_tensor("neggrid", [1, NPAIR * RES], F32, kind="ExternalInput")
    msk_d = nc.dram_tensor("mask112", [G, G], F32, kind="ExternalInput")
    idn_d = nc.dram_tensor("ident112", [G, G], F32, kind="ExternalInput")
    sel_d = nc.dram_tensor("sel4", [4, G], F32, kind="ExternalInput")
    ks_d = nc.dram_tensor("ksum", [G, RES], F32, kind="ExternalInput")
    out_d = nc.dram_tensor("out", [BL, RES, RES], F32, kind="ExternalOutput")

    W = NPAIR * RES          # 896 columns per coordinate
    H = W // 2               # 448 = psum-bank-sized half

    with tile.TileContext(nc) as tc:
        with (
            tc.tile_pool(name="const", bufs=1) as cpool,
            tc.tile_pool(name="work", bufs=1) as wpool,
            tc.tile_pool(name="ebuf", bufs=NCH) as epool,
            tc.tile_pool(name="dxp", bufs=4, space="PSUM") as dxpool,
            tc.tile_pool(name="sp", bufs=3, space="PSUM") as spool,
            tc.tile_pool(name="dram", bufs=1, space="DRAM") as dpool,
        ):
            # ---- constants / inputs to SBUF ----
            basT6 = cpool.tile([PTS + 1, STEPS], F32)
            nc.sync.dma_start(basT6[:], bas_d[:])
            mask112 = cpool.tile([G, G], F32)
            nc.sync.dma_start(mask112[:], msk_d[:])
            ident = cpool.tile([G, G], F32)
            nc.sync.dma_start(ident[:], idn_d[:])
            sel4 = cpool.tile([4, G], F32)
            nc.sync.dma_start(sel4[:], sel_d[:])
            ksum = cpool.tile([G, RES], F32)
            nc.sync.dma_start(ksum[:], ks_d[:])
            zpT = cpool.tile([K, BL], F32)
            nc.sync.dma_start(zpT[:], zpt_d[:])
            zwh = cpool.tile([NPAIR, 3], F32)
            nc.sync.dma_start(zwh[:], zwh_d[:])
            A = cpool.tile([NPAIR, 32], F32)
            nc.gpsimd.memset(A[:], 0.0)
            nc.sync.dma_start(A[:, 0:10], zw10_d[:])

            rhs6x = cpool.tile([PTS + 1, W], F32)
            rhs6y = cpool.tile([PTS + 1, W], F32)
            nc.sync.dma_start(rhs6x[PTS : PTS + 1, :], ngr_d[:])
            nc.sync.dma_start(rhs6y[PTS : PTS + 1, :], ngr_d[:])

            # ---- affine: pts = z_what * s + shift  (pairs on partitions) ----
            B = wpool.tile([NPAIR, 32], F32)
            nc.gpsimd.memset(B[:], 0.0)
            nc.vector.tensor_scalar_mul(B[:, 0:10], A[:, 0:10], zwh[:, 0:1])
            nc.vector.tensor_scalar_add(B[:, 0:5], B[:, 0:5], zwh[:, 1:2])
            nc.vector.tensor_scalar_add(B[:, 5:10], B[:, 5:10], zwh[:, 2:3])
            # Transpose [32, 10] -> two [5, 32] blocks via a DRAM round-trip
            # (strided DMA; keeps compute reads at partition base 0).
            scr = dpool.tile([NPAIR, 10], F32)
            nc.sync.dma_start(scr[:], B[:, 0:10])
            ptsTx = wpool.tile([PTS, NPAIR], F32)
            nc.sync.dma_start(ptsTx[:], scr[:, 0:PTS].rearrange("a b -> b a"))
            ptsTy = wpool.tile([PTS, NPAIR], F32)
            nc.sync.dma_start(ptsTy[:], scr[:, PTS : 2 * PTS].rearrange("a b -> b a"))

            # broadcast each pair's control-point coords across the 28 grid cols
            nc.vector.tensor_copy(
                rhs6x[0:PTS, :].rearrange("p (a b) -> p a b", b=RES),
                ptsTx[0:PTS, :, None].broadcast_to([PTS, NPAIR, RES]),
            )
            nc.vector.tensor_copy(
                rhs6y[0:PTS, :].rearrange("p (a b) -> p a b", b=RES),
                ptsTy[0:PTS, :, None].broadcast_to([PTS, NPAIR, RES]),
            )

            if stage >= 2:
                # ---- per t-chunk: dx via matmul, square, exp ----
                Es = []
                for c in range(NCH):
                    Ec = epool.tile([CL, 2 * W], F32, tag="E")
                    Es.append(Ec)
                    lhs = basT6[:, CL * c : CL * (c + 1)]
                    for coord, rhs6 in ((0, rhs6x), (1, rhs6y)):
                        for h in range(2):
                            dxp = dxpool.tile([CL, H], F32, tag="dx")
                            nc.tensor.matmul(
                                dxp[:], lhs, rhs6[:, H * h : H * (h + 1)],
                                start=True, stop=True,
                            )
                            sl = Ec[:, coord * W + H * h : coord * W + H * (h + 1)]
                            nc.scalar.activation(sl, dxp[:], AF.Square)
                            nc.scalar.activation(sl, sl, AF.Exp, scale=-inv)

            if stage >= 3:
                # ---- strokes: S_g[y,x] = sum_t ey[t,y] * ex[t,x], 4 pairs/group ----
                Ssb = wpool.tile([G, BL, G], F32)
                for g in range(BL):
                    Sg = spool.tile([G, G], F32, tag="S")
                    for c in range(NCH):
                        nc.tensor.matmul(
                            Sg[:],
                            Es[c][:, W + G * g : W + G * (g + 1)],   # ey block (lhsT)
                            Es[c][:, G * g : G * (g + 1)],           # ex block (rhs)
                            start=(c == 0), stop=(c == NCH - 1),
                        )
                    nc.vector.tensor_copy(Ssb[:, g, :], Sg[:])

            if stage >= 4:
                # ---- epilogue: presence gating + max-norm + tanh-norm + k-sum ----
                RM = wpool.tile([G, BL], F32)
                scr = wpool.tile([G, G], F32)
                for g in range(BL):
                    # per-row max of each diagonal 28x28 block (off-diag masked to 0)
                    nc.vector.tensor_tensor(
                        scr[:], Ssb[:, g, :], mask112[:], op=ALU.mult)
                    nc.vector.reduce_max(
                        RM[:, g : g + 1], scr[:], axis=mybir.AxisListType.X)

            if stage >= 5:
                rm_d = dpool.tile([G, BL], F32)
                nc.sync.dma_start(rm_d[:], RM[:])
                RMT = wpool.tile([BL, G], F32)
                nc.sync.dma_start(RMT[:], rm_d[:].rearrange("a b -> b a"))
                m2 = wpool.tile([BL, K], F32)
                nc.vector.reduce_max(
                    m2[:], RMT[:].rearrange("p (i y) -> p i y", y=RES),
                    axis=mybir.AxisListType.X,
                )
                m2_d = dpool.tile([BL, K], F32)
                nc.sync.dma_start(m2_d[:], m2[:])
                m2T = wpool.tile([K, BL], F32)
                nc.sync.dma_start(m2T[:], m2_d[:].rearrange("a b -> b a"))
                r2 = wpool.tile([K, BL], F32)
                nc.vector.tensor_tensor(r2[:], m2T[:], zpT[:], op=ALU.mult)
                nc.vector.tensor_scalar_add(r2[:], r2[:], EPS)
                nc.vector.reciprocal(r2[:], r2[:])
                nc.vector.tensor_tensor(r2[:], r2[:], zpT[:], op=ALU.mult)
                R112 = dxpool.tile([G, BL], F32, tag="dx")
                nc.tensor.matmul(R112[:], sel4[:], r2[:], start=True, stop=True)
                Rsb = wpool.tile([G, BL], F32)
                nc.vector.tensor_copy(Rsb[:], R112[:])

            if stage >= 6:
                Tb = wpool.tile([G, BL, G], F32)
                for g in range(BL):
                    nc.vector.scalar_tensor_tensor(
                        Tb[:, g, :], Ssb[:, g, :], Rsb[:, g : g + 1], mask112[:],
                        op0=ALU.mult, op1=ALU.mult,
                    )
                nc.scalar.activation(
                    Tb[:].rearrange("p a b -> p (a b)"),
                    Tb[:].rearrange("p a b -> p (a b)"),
                    AF.Tanh, scale=float(slope_strk),
                )

            if stage >= 7:
                # k-sum via PE: row-block sum of Tb (off-diagonal blocks are zero),
                # then reduce the 4 stroke column-blocks per batch on DVE.
                ksp = []
                for h in range(2):
                    kp = dxpool.tile([RES, H], F32, tag="dx")
                    ksp.append(kp)
                    nc.tensor.matmul(
                        kp[:], ksum[:], Tb[:].rearrange("p a b -> p (a b)")[:, H * h : H * (h + 1)],
                        start=True, stop=True,
                    )
                at = wpool.tile([RES, BL, RES], F32)
                for h in range(2):
                    # kp columns: (g, j, x) for 4 batches g; sum over stroke j
                    nc.vector.reduce_sum(
                        at[:, 4 * h : 4 * (h + 1), :],
                        ksp[h][:].rearrange("p (g j x) -> p g x j", g=4, j=K),
                        axis=mybir.AxisListType.X,
                    )
                nc.scalar.activation(at[:], at[:], AF.Tanh, scale=float(slope) * post1)
                nc.vector.tensor_scalar_mul(at[:], at[:], post2)

                nc.sync.dma_start(out_d[:].rearrange("b y x -> y b x"), at[:])


            if stage < 7:
                zt = wpool.tile([BL, RES * RES], F32)
                nc.gpsimd.memset(zt[:], 0.0)
                nc.sync.dma_start(out_d[:].rearrange("b y x -> b (y x)"), zt[:])

    nc.compile()
    return nc


_CACHE = {}


def _get_program(sigma, slope_strk, slope):
    key = (float(sigma), float(slope_strk), float(slope))
    if key not in _CACHE:
        _CACHE[key] = _build_program(*key)
    return _CACHE[key]


def kernel(z_pres, z_what, z_where, sigma, slope_strk, slope):
    z_pres = np.asarray(z_pres, np.float32)
    z_what = np.asarray(z_what, np.float32)
    z_where = np.asarray(z_where, np.float32)
    nc = _get_program(float(sigma), float(slope_strk), float(slope))

    basT6, neggrid, mask112, ident112, sel4, ksum = _host_consts()
    in_maps = []
    for c in range(NCORES):
        sl = slice(c * BL, (c + 1) * BL)
        zw = z_what[sl].reshape(NPAIR, PTS, 2)            # [32, 5, 2]
        zw10 = np.ascontiguousarray(
            zw.transpose(0, 2, 1).reshape(NPAIR, 10))     # col = coord*5 + pt
        zwh = np.ascontiguousarray(z_where[sl].reshape(NPAIR, 3))
        zpt = np.ascontiguousarray(z_pres[sl].T)          # [4, 8]
        in_maps.append({
            "zw10": zw10, "zwh": zwh, "zpt": zpt,
            "bast6": np.ascontiguousarray(basT6),
            "neggrid": np.ascontiguousarray(neggrid),
            "mask112": np.ascontiguousarray(mask112),
            "ident112": np.ascontiguousarray(ident112),
            "sel4": np.ascontiguousarray(sel4),
            "ksum": np.ascontiguousarray(ksum),
        })

    res = run_bass_kernel_spmd(nc, in_maps, core_ids=list(range(NCORES)))
    out = np.concatenate([r["out"] for r in res.results], axis=0)  # [64, 28, 28]
    return out[:, None].astype(np.float32)



# revision 2
# speedup vs baseline: 1.0089x; 1.0089x over previous
"""Bezier-stroke rasterizer (AIR/Guide-style) as a Trainium2 Bass/Tile kernel.

Math per (batch, stroke): control points -> Bezier curve -> gaussian blob
rasterization summed along the curve -> presence gating -> max-norm ->
tanh-norm -> sum over strokes -> tanh-norm.

Factorization: exp(-inv*(dy^2+dx^2)) = ey[t,y]*ex[t,x], so the raster is
S = ey^T @ ex contracted over curve samples t on the PE partition dim.

Key performance structure vs the naive version:
- T=128 curve samples (one PE chunk) with Euler-Maclaurin endpoint weights
  folded into the Exp activation bias, matching the 500-sample reference
  sum to ~1e-3 (the maxnorm cancels the sample-density factor).
- dx/dy via a single fp32r matmul per quarter (448 cols -> 1 cycle/row).
- E matrices in fp16: stroke matmuls run at 1 cycle/row.
- Max-norm uses PE transposes (no DRAM round trips); per-stroke scale is
  applied by one broadcast DVE multiply; k-sum runs as PSUM-accumulated
  matmuls against identity slices (no DVE reduction).
- Batch halves (4+4) pipeline through the whole epilogue independently.

Sharding: pure data parallel, 8 batches per core across 8 NeuronCores.
"""

import sys
import numpy as np
from math import comb, tanh, log

sys.path.insert(0, "/opt/trn_rl_repo")

from concourse import bass, bacc, tile, mybir, bass_isa  # noqa: E402
from concourse.bass_utils import run_bass_kernel_spmd  # noqa: E402

BS, K, PTS, RES = 64, 4, 5, 28
T = 128                     # curve samples (contraction dim of stroke matmul)
REF_STEPS = 500             # reference's sample count (for endpoint weights)
NCORES = 8
BL = BS // NCORES           # local batches per core = 8
NPAIR = BL * K              # (batch, stroke) pairs per core = 32
W = NPAIR * RES             # 896 columns per coordinate block
Q = W // 2                  # 448 = one PSUM-bank-sized quarter
G = 4 * RES                 # 112 rows per batch group (4 strokes x 28)
EPS = 1e-6
F32 = mybir.dt.float32
F32R = mybir.dt.float32r
F16 = mybir.dt.float16
AF = mybir.ActivationFunctionType
ALU = mybir.AluOpType
AX = mybir.AxisListType


def _host_consts():
    t = np.linspace(0.0, 1.0, T, dtype=np.float32)[:, None]
    i = np.arange(PTS, dtype=np.float32)[None, :]
    binom = np.array([comb(PTS - 1, j) for j in range(PTS)], dtype=np.float32)[None, :]
    basis = binom * (t**i) * ((1.0 - t) ** (PTS - 1 - i))        # [T, 5]
    grid = np.linspace(0.0, 1.0, RES, dtype=np.float32)          # [28]

    # endpoint weights: the reference sums 500 samples; a T-sample sum
    # underweights interior vs endpoints by the density ratio. w folds the
    # Euler-Maclaurin endpoint correction in; applied as ln(w) bias on ey.
    c = (REF_STEPS - 1) / (T - 1)
    w_end = (c + 1.0) / (2.0 * c)
    lnw = np.zeros((T, 1), np.float32)
    lnw[0, 0] = lnw[-1, 0] = log(w_end)

    ident = np.eye(G, dtype=np.float16)                          # [112, 112]
    return basis, grid, lnw, ident


def _build_program(sigma, slope_strk, slope):
    inv = 1.0 / (2.0 * sigma * sigma)
    post1 = 1.0 / tanh(slope_strk)
    post2 = 1.0 / tanh(slope)

    nc = bacc.Bacc(None, target_bir_lowering=False)

    rhs_d = nc.dram_tensor("rhs6", [6, 128 + 2 * W], F32R, kind="ExternalInput")
    cb_d = nc.dram_tensor("cblob", [T, 33], F32, kind="ExternalInput")
    fb_d = nc.dram_tensor("fblob", [G, G], F16, kind="ExternalInput")
    out_d = nc.dram_tensor("out", [BL, RES, RES], F32, kind="ExternalOutput")

    with tile.TileContext(nc) as tc:
        with (
            tc.tile_pool(name="const", bufs=1) as cpool,
            tc.tile_pool(name="work", bufs=1) as wpool,
            tc.tile_pool(name="dxp", bufs=4, space="PSUM") as dxpool,
            tc.tile_pool(name="sp", bufs=2, space="PSUM") as spool,
        ):
            # ---- inputs / constants ----
            rhs6 = cpool.tile([6, 128 + 2 * W], F32R)
            nc.sync.dma_start(rhs6[:], rhs_d[:])
            cblob = cpool.tile([T, 33], F32)
            nc.scalar.dma_start(cblob[:], cb_d[:])
            fblob = cpool.tile([G, G], F16)
            nc.scalar.dma_start(fblob[:], fb_d[:])

            lnw = cblob[:, 0:1]
            ident = fblob[:, 0:G]
            basT6 = rhs6[:, 0:128]

            # ---- dx/dy quarters: one fp32r matmul each ----
            # quarter q: 0 = x half0, 1 = y half0, 2 = x half1, 3 = y half1
            qoff = [128, 128 + W, 128 + Q, 128 + W + Q]
            dxp = []
            for q in range(4):
                p = dxpool.tile([T, Q], F32, tag="dx", name=f"dxp{q}")
                dxp.append(p)
                nc.tensor.matmul(
                    p[:], basT6, rhs6[:, qoff[q] : qoff[q] + Q],
                    start=True, stop=True,
                )

            # ---- square + exp (fp16 out). Act owns h0 squares; h1
            # squares run on DVE via SBUF staging so Act's serial chain
            # shortens and strokes-h1 unblocks earlier.
            E = wpool.tile([T, 2 * W], F16)
            dxs = wpool.tile([T, 2 * Q], F16)
            eoff = [0, W, Q, W + Q]
            nc.scalar.activation(dxp[0][:], dxp[0][:], AF.Square)
            nc.scalar.activation(dxp[1][:], dxp[1][:], AF.Square)
            nc.scalar.activation(
                E[:, eoff[0] : eoff[0] + Q], dxp[0][:], AF.Exp, scale=-inv)
            nc.scalar.activation(
                E[:, eoff[1] : eoff[1] + Q], dxp[1][:], AF.Exp, scale=-inv,
                bias=lnw)
            nc.vector.tensor_copy(dxs[:, 0:Q], dxp[2][:])
            nc.vector.tensor_tensor(
                dxs[:, 0:Q], dxs[:, 0:Q], dxs[:, 0:Q], op=ALU.mult)
            nc.vector.tensor_copy(dxs[:, Q : 2 * Q], dxp[3][:])
            nc.vector.tensor_tensor(
                dxs[:, Q : 2 * Q], dxs[:, Q : 2 * Q], dxs[:, Q : 2 * Q],
                op=ALU.mult)
            nc.scalar.activation(
                E[:, eoff[2] : eoff[2] + Q], dxs[:, 0:Q], AF.Exp, scale=-inv)
            nc.scalar.activation(
                E[:, eoff[3] : eoff[3] + Q], dxs[:, Q : 2 * Q], AF.Exp,
                scale=-inv, bias=lnw)

            # ---- stroke matmuls: S_h[(k,y),(g,k',x)] = ey^T @ ex ----
            S = []
            for h in range(2):
                sh = spool.tile([G, 4 * G], F32, tag="S")
                S.append(sh)
                for gg in range(4):
                    g = 4 * h + gg
                    nc.tensor.matmul(
                        sh[:, G * gg : G * (gg + 1)],
                        E[:, W + G * g : W + G * (g + 1)],
                        E[:, G * g : G * (g + 1)],
                        start=True, stop=True,
                    )

            # ---- epilogue per batch-half ----
            # maxnorm scale: x-max -> +EP (diag +eps/zp, off-diag -BIG: folds
            # mask and eps; max is monotonic) -> cross-partition max (gpsimd
            # ucode all-reduce: every partition gets m_{g,j}) -> reciprocal.
            # Then one broadcast multiply gates S, tanh folds slope_strk.
            BM = [wpool.tile([G, 16], F32, tag=f"bm{h}", name=f"BM{h}") for h in range(2)]
            Tpre = [wpool.tile([G, 4 * G], F32, tag=f"tp{h}", name=f"Tpre{h}") for h in range(2)]
            Tb = [wpool.tile([G, 4 * G], F16, tag=f"tb{h}", name=f"Tb{h}") for h in range(2)]
            at = [wpool.tile([RES, G], F32, tag=f"at{h}", name=f"at{h}") for h in range(2)]
            img = [None, None]

            def bmax(h):
                nc.vector.reduce_max(
                    BM[h][:].rearrange("p (g j) -> p g j", j=K),
                    S[h][:].rearrange("p (g j x) -> p g j x", j=K, x=RES),
                    axis=AX.X,
                )
                nc.vector.tensor_tensor(
                    BM[h][:], BM[h][:], cblob[0:G, 1 + 16 * h : 17 + 16 * h],
                    op=ALU.add,
                )

            def rmax(h):
                nc.gpsimd.partition_all_reduce(
                    BM[h][:], BM[h][:], G, bass_isa.ReduceOp.max)

            def gate(h, splits=1):
                nc.vector.reciprocal(BM[h][:], BM[h][:])
                # h0's gate may split so it can fill DVE gaps between the
                # h1 chain's ops instead of delaying them.
                step = 4 // splits
                for s in range(splits):
                    cs, ce = s * step, (s + 1) * step
                    nc.vector.tensor_tensor(
                        Tpre[h][:, G * cs : G * ce].rearrange(
                            "p (g j x) -> p g j x", j=K, x=RES),
                        S[h][:, G * cs : G * ce].rearrange(
                            "p (g j x) -> p g j x", j=K, x=RES),
                        BM[h][:, 4 * cs : 4 * ce].rearrange(
                            "p (g j) -> p g j", j=K)[
                            :, :, :, None].broadcast_to([G, step, K, RES]),
                        op=ALU.mult,
                    )

            def tanh_ksum(h):
                nc.scalar.activation(
                    Tb[h][:], Tpre[h][:], AF.Tanh, scale=float(slope_strk))
                img[h] = dxpool.tile([RES, G], F32, tag="dx", name=f"img{h}")
                for j in range(K):
                    nc.tensor.matmul(
                        img[h][:],
                        ident[:, RES * j : RES * (j + 1)],
                        Tb[h][:].rearrange(
                            "p (g j x) -> p g j x", j=K, x=RES)[:, :, j : j + 1, :],
                        start=(j == 0), stop=(j == K - 1),
                    )

            def finish(h):
                nc.scalar.activation(
                    at[h][:], img[h][:], AF.Tanh, scale=float(slope) * post1)
                nc.vector.tensor_scalar_mul(at[h][:], at[h][:], post2)
                eng = nc.sync if h == 0 else nc.scalar
                eng.dma_start(
                    out_d[4 * h : 4 * h + 4].rearrange("b y x -> y b x"),
                    at[h][:].rearrange("p (b x) -> p b x", x=RES),
                )

            bmax(0)
            rmax(0)
            bmax(1)
            gate(0, splits=2)
            rmax(1)
            gate(1)
            tanh_ksum(0)
            tanh_ksum(1)
            finish(0)
            finish(1)

    nc.compile()
    return nc


_CACHE = {}


def _get_program(sigma, slope_strk, slope):
    key = (float(sigma), float(slope_strk), float(slope))
    if key not in _CACHE:
        _CACHE[key] = _build_program(*key)
    return _CACHE[key]


def _host_inputs(z_pres, z_what, z_where):
    basis, grid, lnw, ident = _host_consts()
    fblob = ident

    in_maps = []
    for c in range(NCORES):
        sl = slice(c * BL, (c + 1) * BL)
        zw = z_what[sl].reshape(NPAIR, PTS, 2)                   # [32, 5, 2]
        zwh = z_where[sl].reshape(NPAIR, 3)
        zp = z_pres[sl]                                          # [8, 4]
        s = zwh[:, 0:1]
        pts = zw * s[:, :, None] + zwh[:, None, 1:3]
        ptsx = pts[:, :, 0]                                      # [32, 5]
        ptsy = pts[:, :, 1]

        rhs6 = np.zeros((6, 128 + 2 * W), np.float32)
        rhs6[:5, 0:128] = basis.T
        rhs6[5, 0:128] = 1.0
        for blk, p5 in ((0, ptsx), (1, ptsy)):
            off = 128 + blk * W
            rhs6[:5, off : off + W] = np.repeat(p5.T, RES, axis=1)
            rhs6[5, off : off + W] = -np.tile(grid, NPAIR)

        cblob = np.zeros((T, 33), np.float32)
        cblob[:, 0:1] = lnw
        # EP_h[(k,y),(g,j)]: +eps/zp on diagonal stroke blocks (j==k), -BIG
        # off-diagonal -- one add replaces the mask-mult and the eps-add.
        epszp = EPS / np.maximum(zp, 1e-37)                      # [8, 4]
        kidx = np.arange(G) // RES                               # [112]
        diag = (kidx[:, None] == np.arange(K)[None, :])          # [112, 4]
        for h in range(2):
            ep = np.where(diag[:, None, :],
                          epszp[4 * h : 4 * h + 4][None, :, :],
                          np.float32(-1e30))                     # [112, 4, 4]
            cblob[0:G, 1 + 16 * h : 17 + 16 * h] = ep.reshape(G, 16)

        in_maps.append({
            "rhs6": np.ascontiguousarray(rhs6),
            "cblob": np.ascontiguousarray(cblob),
            "fblob": np.ascontiguousarray(fblob),
        })
    return in_maps


def kernel(z_pres, z_what, z_where, sigma, slope_strk, slope):
    z_pres = np.asarray(z_pres, np.float32)
    z_what = np.asarray(z_what, np.float32)
    z_where = np.asarray(z_where, np.float32)
    nc = _get_program(float(sigma), float(slope_strk), float(slope))
    in_maps = _host_inputs(z_pres, z_what, z_where)
    res = run_bass_kernel_spmd(nc, in_maps, core_ids=list(range(NCORES)))
    out = np.concatenate([r["out"] for r in res.results], axis=0)
    return out[:, None].astype(np.float32)
